# revision 17
# baseline (speedup 1.0000x reference)
"""Trainium2 Bass kernel for nn_MaxBPLayer (max-product belief propagation layer).

Exploits the deterministic graph structure of reference._build_graph():
  edge e = 3f + d  (edges grouped by factor), edge_var[3f+d] = ((f%V) + d*A) % V.
With f = k*V + j*A + u (k in {0,1}, j in {0,1,2}, u in [0,A)), every step of the
layer is diagonal in u, so u is sharded across the 8 NeuronCores with zero
communication. On-chip layout: u -> (partition p, column q), channels in the
free dimension. All gathers/scatters become strided/broadcast access patterns.

Per-column channel math is validated against the reference in golden.py.
"""

import sys

sys.path.insert(0, "/opt/trn_rl_repo")

from contextlib import ExitStack

import numpy as np

import concourse.bass as bass
import concourse.tile as tile
from concourse import mybir
from concourse.bass_utils import run_bass_kernel_spmd

# ---- problem constants (hardcoded per task instructions) ----
NCORES = 8
V = 300000
D = 3
C = 2
A = V // D            # 100000
F_ = 2 * V            # 600000
E = 3 * F_            # 1800000
CHUNK = A // NCORES   # 12500 u-values per core
P = 128
QT = 98               # CHUNK padded to P*QT = 12544
NPAD = P * QT
NQ = 2                # q-tiles per core
X = QT // NQ          # columns per tile

DT = mybir.dt.float32
ALU = mybir.AluOpType
AF = mybir.ActivationFunctionType


def _split_multi_waits(nc):
    """This container's walrus accepts at most ONE sem wait per instruction.
    Split any multi-wait sync_info into preceding single-wait NoOps."""
    cnt = 0
    for bb in nc.main_func.blocks:
        insts = list(bb.instructions)
        out, changed = [], False
        for inst in insts:
            si = inst.sync_info
            if si is not None and len(si.on_wait) > 1:
                changed = True
                waits = list(si.on_wait)
                for w in waits[:-1]:
                    cnt += 1
                    nop = mybir.InstNoOp(name=f"wsplit-{cnt}", ins=[], outs=[])
                    nop.engine = inst.engine
                    nop.sync_info = mybir.SyncInfo(on_wait=[w], on_update=[])
                    out.append(nop)
                inst.sync_info = mybir.SyncInfo(
                    on_wait=[waits[-1]], on_update=list(si.on_update)
                )
            out.append(inst)
        if changed:
            bb.instructions = out
    return cnt


def _ap(tile_ap, off_ch, dims, x):
    """Custom strided AP into a [128, nch*x] SBUF tile.

    off_ch: starting channel; dims: list of (step_ch_float_ok, num) where step is
    in CHANNELS (may be 0 for broadcast); innermost dim gets (1, span_elems).
    """
    base = tile_ap  # AP over full tile [:, :]
    part = list(base.ap)[0]  # [stride, 128] partition pair
    *outer, (ls, ln) = dims
    assert ls == 1, "innermost dim must be contiguous"
    free = [[int(s * x), int(n)] for s, n in outer] + [[1, int(ln * x)]]
    return bass.AP(base.tensor, int(off_ch * x), [list(part)] + free)


def _build_nc(x=X, nq=NQ, split=True):
    nc = bass.Bass(trn_type="TRN2")
    m0_d = nc.dram_tensor("m0", [P, nq * 36 * x], DT, kind="ExternalInput").ap()
    pot_d = nc.dram_tensor("pot", [P, nq * 48 * x], DT, kind="ExternalInput").ap()
    nf_d = nc.dram_tensor("nf", [P, nq * 36 * x], DT, kind="ExternalOutput").ap()
    vb2_d = nc.dram_tensor("vb2", [P, nq * 6 * x], DT, kind="ExternalOutput").ap()
    fb2_d = nc.dram_tensor("fb2", [P, nq * 48 * x], DT, kind="ExternalOutput").ap()

    with tile.TileContext(nc) as tc:
        with ExitStack() as ctx:
            p2 = ctx.enter_context(tc.tile_pool(name="p2", bufs=2))
            p3 = ctx.enter_context(tc.tile_pool(name="p3", bufs=3))
            p4 = ctx.enter_context(tc.tile_pool(name="p4", bufs=4))
            for t in range(nq):
                _tile_body(nc, p2, p3, p4, t, x, m0_d, pot_d, nf_d, vb2_d, fb2_d)
    if split:
        _split_multi_waits(nc)
    return nc


def _vb_block(nc, p2, p3, x, msg, gext, vbt, vbe, eng):
    """vb fold: gext = [g; g], vbe = [vb; vb] where
    g[j,d,c] = msg[k0]+msg[k1], vb[j,c] = sum_d g[(j-d)%3, d, c]."""
    gf, mf = gext[:], msg[:]
    nc.vector.tensor_tensor(gf[:, : 18 * x], mf[:, : 18 * x], mf[:, 18 * x :], ALU.add)
    nc.scalar.copy(gf[:, 18 * x :], gf[:, : 18 * x])

    def term(d):
        # channels (j+3-d)*6 + 2d + c, j=0..2: (j: 6ch, 3)(cq fold: 2x)
        return _ap(gf, (3 - d) * 6 + 2 * d, [(6, 3), (1, 2)], x)

    eng.tensor_tensor(vbt[:], term(0), term(1), ALU.add)
    eng.tensor_tensor(vbe[:][:, : 6 * x], vbt[:], term(2), ALU.add)
    nc.scalar.copy(vbe[:][:, 6 * x :], vbe[:][:, : 6 * x])


def _v2f_block(nc, x, vbe, msg, v2f):
    """v2f[k,j,d,c] = vbe[2(j+d)+c] - msg[k,j,d,c]; one instr per k via the
    overlapping (j: 2ch)(d: 2ch) AP."""
    gather = _ap(vbe[:], 0, [(2, 3), (2, 3), (1, 2)], x)
    for k in (0, 1):
        out = _ap(v2f[:], 18 * k, [(6, 3), (2, 3), (1, 2)], x)
        in1 = _ap(msg[:], 18 * k, [(6, 3), (2, 3), (1, 2)], x)
        eng = nc.vector if k == 0 else nc.gpsimd
        eng.tensor_tensor(out, gather, in1, ALU.subtract)


def _t1_block(nc, x, v2f, t1, eng):
    """t1[kj,a,b] = v2f[kj,0,a] + v2f[kj,1,b]; split by a."""
    for a in (0, 1):
        out = _ap(t1[:], 2 * a, [(4, 6), (1, 2)], x)          # (kj)(bq)
        in0 = _ap(v2f[:], a, [(6, 6), (0, 2), (1, 1)], x)     # bcast over b
        in1 = _ap(v2f[:], 2, [(6, 6), (1, 2)], x)             # (kj)(bq)
        eng.tensor_tensor(out, in0, in1, ALU.add)


def _potplus_block(nc, x, pot, t1, out48, engines):
    """out48[kj,c,ab] = pot + t1[kj,ab] (bcast over c); split by c.
    pot/out48 are in (kj, c, ab) channel order -> 2-free-dim APs (Pool-safe)."""
    for c, eng in zip((0, 1), engines):
        out = _ap(out48[:], 4 * c, [(8, 6), (1, 4)], x)
        in0 = _ap(pot[:], 4 * c, [(8, 6), (1, 4)], x)
        in1 = _ap(t1[:], 0, [(4, 6), (1, 4)], x)
        eng.tensor_tensor(out, in0, in1, ALU.add)


def _tile_body(nc, p2, p3, p4, t, x, m0_d, pot_d, nf_d, vb2_d, fb2_d):
    tsl = lambda nch: slice(t * nch * x, (t + 1) * nch * x)

    m0 = p2.tile([P, 36 * x], DT, tag="m0")
    pot = p2.tile([P, 48 * x], DT, tag="pot")
    nc.sync.dma_start(out=m0[:], in_=m0_d[:, tsl(36)])
    nc.sync.dma_start(out=pot[:], in_=pot_d[:, tsl(48)])

    gext = p2.tile([P, 36 * x], DT, tag="gext")
    vbt = p2.tile([P, 6 * x], DT, tag="vbt")
    vbe = p3.tile([P, 12 * x], DT, tag="vbe")
    v2f = p2.tile([P, 36 * x], DT, tag="v2f")
    _vb_block(nc, p2, p3, x, m0, gext, vbt, vbe, nc.gpsimd)
    _v2f_block(nc, x, vbe, m0, v2f)

    # ---- round-1 max-marginal block: t[kj,d,c] directly (fb cancels) ----
    T1 = p4.tile([P, 24 * x], DT, tag="s24")
    _t1_block(nc, x, v2f, T1, nc.vector)

    Q = p3.tile([P, 48 * x], DT, tag="q48")
    # Q[kj,c,ab] = pot + v2f[kj,2,c] bcast over ab; split by c (pot is (kj,c,ab))
    for c in (0, 1):
        nc.vector.tensor_tensor(
            _ap(Q[:], 4 * c, [(8, 6), (1, 4)], x),
            _ap(pot[:], 4 * c, [(8, 6), (1, 4)], x),
            _ap(v2f[:], 4 + c, [(6, 6), (0, 4), (1, 1)], x),
            ALU.add,
        )
    R = p4.tile([P, 24 * x], DT, tag="s24")
    nc.vector.tensor_tensor(
        R[:],
        _ap(Q[:], 0, [(8, 6), (1, 4)], x),
        _ap(Q[:], 4, [(8, 6), (1, 4)], x),
        ALU.max,
    )
    tt = p4.tile([P, 36 * x], DT, tag="t36")
    # S1[kj,a,b] = R + v2f[kj,1,b]; t[kj,0,a] = max_b S1
    S1 = p4.tile([P, 24 * x], DT, tag="s24")
    nc.vector.tensor_tensor(
        S1[:],
        R[:],
        _ap(v2f[:], 2, [(6, 6), (0, 2), (1, 2)], x),
        ALU.add,
    )
    nc.vector.tensor_tensor(
        _ap(tt[:], 0, [(6, 6), (1, 2)], x),
        _ap(S1[:], 0, [(4, 6), (2, 2), (1, 1)], x),
        _ap(S1[:], 1, [(4, 6), (2, 2), (1, 1)], x),
        ALU.max,
    )
    # S0[kj,a,b] = R + v2f[kj,0,a] (bcast b; split by a); t[kj,1,b] = max_a S0
    S0 = p4.tile([P, 24 * x], DT, tag="s24")
    for a in (0, 1):
        nc.vector.tensor_tensor(
            _ap(S0[:], 2 * a, [(4, 6), (1, 2)], x),
            _ap(R[:], 2 * a, [(4, 6), (1, 2)], x),
            _ap(v2f[:], a, [(6, 6), (0, 2), (1, 1)], x),
            ALU.add,
        )
    nc.vector.tensor_tensor(
        _ap(tt[:], 2, [(6, 6), (1, 2)], x),
        _ap(S0[:], 0, [(4, 6), (1, 2)], x),
        _ap(S0[:], 2, [(4, 6), (1, 2)], x),
        ALU.max,
    )
    # Z[kj,c,ab] = pot + T1[kj,ab] (bcast c); m1[kj,c,b] = max_a Z; t[kj,2,c] = max_b m1
    Z = p3.tile([P, 48 * x], DT, tag="q48")
    _potplus_block(nc, x, pot, T1, Z, (nc.gpsimd, nc.gpsimd))
    m1 = p4.tile([P, 24 * x], DT, tag="s24")
    nc.vector.tensor_tensor(
        m1[:],
        _ap(Z[:], 0, [(8, 6), (4, 2), (1, 2)], x),
        _ap(Z[:], 2, [(8, 6), (4, 2), (1, 2)], x),
        ALU.max,
    )
    nc.vector.tensor_tensor(
        _ap(tt[:], 4, [(6, 6), (1, 2)], x),
        _ap(m1[:], 0, [(4, 6), (2, 2), (1, 1)], x),
        _ap(m1[:], 1, [(4, 6), (2, 2), (1, 1)], x),
        ALU.max,
    )

    # ---- lse over c pairs: nf = t - ln(exp(t0)+exp(t1)) ----
    ex = p4.tile([P, 36 * x], DT, tag="t36")
    nc.scalar.activation(ex[:], tt[:], AF.Exp)
    s18 = p2.tile([P, 18 * x], DT, tag="s18")
    nc.gpsimd.tensor_tensor(
        s18[:],
        _ap(ex[:], 0, [(2, 18), (1, 1)], x),
        _ap(ex[:], 1, [(2, 18), (1, 1)], x),
        ALU.add,
    )
    l18 = p2.tile([P, 18 * x], DT, tag="l18")
    nc.scalar.activation(l18[:], s18[:], AF.Ln)
    nf = p3.tile([P, 36 * x], DT, tag="t36")
    for c in (0, 1):
        nc.vector.tensor_tensor(
            _ap(nf[:], c, [(2, 18), (1, 1)], x),
            _ap(tt[:], c, [(2, 18), (1, 1)], x),
            l18[:],
            ALU.subtract,
        )

    # ---- round 2 ----
    g2 = p2.tile([P, 36 * x], DT, tag="gext")
    vbt2 = p2.tile([P, 6 * x], DT, tag="vbt")
    vb2e = p3.tile([P, 12 * x], DT, tag="vbe")
    v2f2 = p2.tile([P, 36 * x], DT, tag="v2f")
    _vb_block(nc, p2, p3, x, nf, g2, vbt2, vb2e, nc.gpsimd)
    _v2f_block(nc, x, vb2e, nf, v2f2)

    T1p = p4.tile([P, 24 * x], DT, tag="s24")
    _t1_block(nc, x, v2f2, T1p, nc.vector)
    fb2a = p3.tile([P, 48 * x], DT, tag="q48")
    _potplus_block(nc, x, pot, T1p, fb2a, (nc.gpsimd, nc.vector))
    fb2 = p2.tile([P, 48 * x], DT, tag="fb2")
    # fb2[kj,c,ab] = fb2a + v2f2[kj,2,c] bcast over ab; split by c
    for c in (0, 1):
        nc.vector.tensor_tensor(
            _ap(fb2[:], 4 * c, [(8, 6), (1, 4)], x),
            _ap(fb2a[:], 4 * c, [(8, 6), (1, 4)], x),
            _ap(v2f2[:], 4 + c, [(6, 6), (0, 4), (1, 1)], x),
            ALU.add,
        )

    nc.sync.dma_start(out=nf_d[:, tsl(36)], in_=nf[:])
    nc.sync.dma_start(out=vb2_d[:, tsl(6)], in_=vb2e[:][:, : 6 * x])
    nc.sync.dma_start(out=fb2_d[:, tsl(48)], in_=fb2[:])


# ---------------- host side ----------------

_NC_CACHE = {}


def _install_ntff_hook():
    """Recreate antenv.axon_hooks (absent in this image) so trace=True works."""
    import sys as _s
    import types

    if "antenv.axon_hooks" in _s.modules:
        return
    hook = None
    try:
        if "/root/.axon_site" not in _s.path:
            _s.path.insert(0, "/root/.axon_site")
        from trn_agent_boot.trn_boot import _ntff_profile_via_ctypes

        hook = _ntff_profile_via_ctypes("/opt/axon/libaxon_pjrt.so")
    except Exception:
        hook = None
    mod = types.ModuleType("antenv.axon_hooks")
    mod.get_axon_ntff_profile_hook = lambda: hook
    mod.set_axon_ntff_profile_hook = lambda h: None
    _s.modules["antenv.axon_hooks"] = mod

    import concourse.bass_utils as _bu

    orig = _bu.upload_artifacts

    def _safe_upload(tmpdir):
        try:
            return orig(tmpdir)
        except Exception:
            return "local://" + tmpdir

    _bu.upload_artifacts = _safe_upload


def _get_nc():
    key = (X, NQ)
    if key not in _NC_CACHE:
        _NC_CACHE[key] = _build_nc()
    return _NC_CACHE[key]


def _pack_core(arr_ch, nch):
    """[nch, CHUNK] -> [P, NQ*nch*X] in per-tile-contiguous layout."""
    buf = np.zeros((nch, NPAD), np.float32)
    buf[:, :CHUNK] = arr_ch
    # (ch, p, t, ql) -> (p, t, ch, ql)
    return (
        buf.reshape(nch, P, NQ, X).transpose(1, 2, 0, 3).reshape(P, NQ * nch * X)
    )


def _unpack_core(buf, nch):
    """[P, NQ*nch*X] -> [nch, CHUNK]"""
    return (
        buf.reshape(P, NQ, nch, X)
        .transpose(2, 0, 1, 3)
        .reshape(nch, NPAD)[:, :CHUNK]
    )


def kernel(
    prv_factorToVar_messages,
    factor_potentials,
    edge_fac=None,
    edge_var=None,
    scatter_idx=None,
    facStates_to_varIdx=None,
    num_vars=None,
    num_factors=None,
    _trace=False,
):
    prv = np.asarray(prv_factorToVar_messages, dtype=np.float32)
    pot = np.asarray(factor_potentials, dtype=np.float32)

    # channel-major views over the full u axis; pot in (k,j,c,a,b) order
    m0_ch = prv.reshape(2, 3, A, 3, 2).transpose(0, 1, 3, 4, 2).reshape(36, A)
    pot_ch = (
        pot.reshape(2, 3, A, 2, 2, 2).transpose(0, 1, 5, 3, 4, 2).reshape(48, A)
    )

    in_maps = []
    for i in range(NCORES):
        sl = slice(i * CHUNK, (i + 1) * CHUNK)
        in_maps.append(
            {
                "m0": _pack_core(m0_ch[:, sl], 36),
                "pot": _pack_core(pot_ch[:, sl], 48),
            }
        )

    nc = _get_nc()
    if _trace:
        _install_ntff_hook()
    res = run_bass_kernel_spmd(
        nc, in_maps, core_ids=list(range(NCORES)), trace=_trace
    )

    nf_ch = np.empty((36, A), np.float32)
    vb2_ch = np.empty((6, A), np.float32)
    fb2_ch = np.empty((48, A), np.float32)
    for i in range(NCORES):
        sl = slice(i * CHUNK, (i + 1) * CHUNK)
        out = res.results[i]
        nf_ch[:, sl] = _unpack_core(out["nf"], 36)
        vb2_ch[:, sl] = _unpack_core(out["vb2"], 6)
        fb2_ch[:, sl] = _unpack_core(out["fb2"], 48)

    f2v = (
        nf_ch.reshape(2, 3, 3, 2, A).transpose(0, 1, 4, 2, 3).reshape(E, 2)
    )
    var_beliefs = vb2_ch.reshape(3, 2, A).transpose(0, 2, 1).reshape(V, 2)
    # fb2 channels are (k, j, c, a, b) -> fac_beliefs[f, a, b, c]
    fac_beliefs = (
        fb2_ch.reshape(2, 3, 2, 2, 2, A)
        .transpose(0, 1, 5, 3, 4, 2)
        .reshape(F_, 2, 2, 2)
    )
    if _trace:
        return (f2v, var_beliefs, fac_beliefs), res
    return f2v, var_beliefs, fac_beliefs
